# revision 5
# baseline (speedup 1.0000x reference)
"""Distributed multi-head causal attention for Trainium2 (8 NeuronCores).

Problem: nn_Attention (B=2, S=2048, D=1024, H=16, DK=DV=64), f32 inputs.
Sharding: batch x head-group. Core c handles batch b=c//4, heads 4*(c%4)..4*(c%4)+3.
Each core: projections (bf16 matmuls, f32 PSUM), scoresT = kh^T-layout scores,
exp with fused 1/sqrt(dk) scale + key-padding bias, causal handled by skipping
above-diagonal tiles + one triangular multiply per diagonal block, PV with an
appended ones-column producing the softmax denominator for free, on-device
normalization (reciprocal + partition broadcast), q-mask fused into the
reciprocal. Host does layout prep (transposes/slices), the gather, and patches
the data-dependent degenerate rows (queries whose entire causal window is
key-masked — the reference's +/-1e10 additive-mask arithmetic makes those rows
attend uniformly to *future* unmasked keys, which the causal-skipping device
kernel intentionally does not compute).
"""

import numpy as np

import concourse.bass as bass
import concourse.mybir as mybir
import concourse.tile as tile
from concourse import bacc
from concourse.bass_utils import run_bass_kernel_spmd

F32 = mybir.dt.float32
BF16 = mybir.dt.bfloat16

MAX = 1e10
B, S, D = 2, 2048, 1024
H, DK, DV = 16, 64, 64
HPC = 4            # heads per core
GW = HPC * DK      # 256: projected width per core
KC = D // 128      # 8 contraction chunks
NKT = S // 128     # 16 key tiles
QCP = 1024         # q chunk width for exp/PV psum tiles
NQC = S // QCP     # 2
VW = DV + 1        # 65: ones column + value dims


def _segs(off, end):
    """512-aligned segments of [off, end) — PSUM-bank-safe matmul pieces."""
    j = off
    while j < end:
        nxt = min(end, (j // 512 + 1) * 512)
        yield j, nxt - j
        j = nxt


def _build():
    nc = bacc.Bacc("TRN2", target_bir_lowering=False, debug=False, num_devices=8)

    qT = nc.dram_tensor("qT", [128, KC * S], F32, kind="ExternalInput").ap()
    kT = nc.dram_tensor("kT", [128, KC * S], F32, kind="ExternalInput").ap()
    vT = nc.dram_tensor("vT", [128, KC * S], F32, kind="ExternalInput").ap()
    wq = nc.dram_tensor("wq", [128, KC * GW], F32, kind="ExternalInput").ap()
    wk = nc.dram_tensor("wk", [128, KC * GW], F32, kind="ExternalInput").ap()
    wv = nc.dram_tensor("wv", [128, KC * GW], F32, kind="ExternalInput").ap()
    kbias = nc.dram_tensor("kbias", [128, NKT], F32, kind="ExternalInput").ap()
    tri = nc.dram_tensor("tri", [128, 128], F32, kind="ExternalInput").ap()
    qmask = nc.dram_tensor("qmask", [1, S], F32, kind="ExternalInput").ap()
    out = nc.dram_tensor("out", [GW, S], F32, kind="ExternalOutput").ap()

    with tile.TileContext(nc) as tc:
        with tc.tile_pool(name="pers", bufs=1) as pers:
            # bf16 copies of inputs (cast during SWDGE DMA)
            qT_sb = pers.tile([128, KC, S], BF16)
            kT_sb = pers.tile([128, KC, S], BF16)
            vT_sb = pers.tile([128, KC, S], BF16)
            for kc in range(KC):
                nc.gpsimd.dma_start(qT_sb[:, kc, :], qT[:, kc * S:(kc + 1) * S])
                nc.gpsimd.dma_start(kT_sb[:, kc, :], kT[:, kc * S:(kc + 1) * S])
                nc.gpsimd.dma_start(vT_sb[:, kc, :], vT[:, kc * S:(kc + 1) * S])
            wq_sb = pers.tile([128, KC, GW], BF16)
            nc.gpsimd.dma_start(wq_sb[:], wq[:, :].rearrange("p (kc n) -> p kc n", kc=KC))
            wk_sb = pers.tile([128, KC, GW], BF16)
            nc.gpsimd.dma_start(wk_sb[:], wk[:, :].rearrange("p (kc n) -> p kc n", kc=KC))
            wv_sb = pers.tile([128, KC, GW], BF16)
            nc.gpsimd.dma_start(wv_sb[:], wv[:, :].rearrange("p (kc n) -> p kc n", kc=KC))
            kbias_sb = pers.tile([128, NKT], F32)
            nc.sync.dma_start(kbias_sb[:], kbias[:, :])
            tri_sb = pers.tile([128, 128], BF16)
            nc.gpsimd.dma_start(tri_sb[:], tri[:, :])
            qmask_sb = pers.tile([1, S], F32)
            nc.sync.dma_start(qmask_sb[:], qmask[:, :])

            # projected tensors
            qhT_sb = pers.tile([128, 2, S], BF16)    # [p, m, s]: qh[s, m*128+p]
            khT_sb = pers.tile([128, 2, S], BF16)
            vh_sb = pers.tile([128, NKT, HPC, VW], BF16)  # col DV = ones
            nc.gpsimd.memset(vh_sb[:, :, :, DV:VW], 1.0)

            # ---- q/k projections: qhT[m*128+p, s] accumulated over kc ----
            with tc.tile_pool(name="ps_qk", bufs=8, space="PSUM") as ps_qk:
                for src_sb, dst_sb in ((qT_sb, qhT_sb), (kT_sb, khT_sb)):
                    tiles = {}
                    for m in range(2):
                        for qc in range(4):
                            tiles[(m, qc)] = ps_qk.tile([128, 512], F32, tag="qk", name=f"qkps_{m}_{qc}")
                    for kc in range(KC):
                        for m in range(2):
                            for qc in range(4):
                                nc.tensor.matmul(
                                    tiles[(m, qc)][:],
                                    wq_sb[:, kc, m * 128:(m + 1) * 128]
                                    if src_sb is qT_sb
                                    else wk_sb[:, kc, m * 128:(m + 1) * 128],
                                    src_sb[:, kc, qc * 512:(qc + 1) * 512],
                                    start=(kc == 0),
                                    stop=(kc == KC - 1),
                                )
                    for m in range(2):
                        for qc in range(4):
                            nc.vector.tensor_copy(
                                dst_sb[:, m, qc * 512:(qc + 1) * 512],
                                tiles[(m, qc)][:],
                            )

            # ---- v projection: vh[st*128+t, j] ----
            with tc.tile_pool(name="ps_v", bufs=4, space="PSUM") as ps_v:
                for st in range(NKT):
                    pv = ps_v.tile([128, GW], F32, tag="v")
                    for kc in range(KC):
                        nc.tensor.matmul(
                            pv[:],
                            vT_sb[:, kc, st * 128:(st + 1) * 128],
                            wv_sb[:, kc, :],
                            start=(kc == 0),
                            stop=(kc == KC - 1),
                        )
                    nc.vector.tensor_copy(
                        vh_sb[:, st, :, 0:DV],
                        pv[:].rearrange("p (h d) -> p h d", d=DV),
                    )

            # ---- attention ----
            with (
                tc.tile_pool(name="ps_s", bufs=2, space="PSUM") as ps_s,
                tc.tile_pool(name="ps_pv", bufs=2, space="PSUM") as ps_pv,
                tc.tile_pool(name="att", bufs=1) as att,
            ):
                for h in range(HPC):
                    p0 = (h % 2) * 64
                    m = h // 2
                    pv_tiles = [ps_pv.tile([VW, QCP], F32, tag="pv", name=f"pvps_{h}_{i}") for i in range(NQC)]
                    for kt in range(NKT):
                        qcp_lo = (kt * 128) // QCP
                        for qcp in range(qcp_lo, NQC):
                            off = max(0, kt * 128 - qcp * QCP)
                            s_ps = ps_s.tile([128, QCP], F32, tag="s")
                            for j0, w in _segs(off, QCP):
                                nc.tensor.matmul(
                                    s_ps[:, j0:j0 + w],
                                    khT_sb[p0:p0 + 64, m, kt * 128:(kt + 1) * 128],
                                    qhT_sb[p0:p0 + 64, m, qcp * QCP + j0:qcp * QCP + j0 + w],
                                    start=True,
                                    stop=True,
                                )
                            p_sb = att.tile([128, QCP], BF16, tag="probs", bufs=4)
                            nc.scalar.activation(
                                p_sb[:, off:QCP],
                                s_ps[:, off:QCP],
                                mybir.ActivationFunctionType.Exp,
                                bias=kbias_sb[:, kt:kt + 1],
                                scale=float(1.0 / np.sqrt(DK)),
                            )
                            if kt * 128 >= qcp * QCP:
                                nc.vector.tensor_mul(
                                    p_sb[:, off:off + 128],
                                    p_sb[:, off:off + 128],
                                    tri_sb[:],
                                )
                            for j0, w in _segs(off, QCP):
                                nc.tensor.matmul(
                                    pv_tiles[qcp][:, j0:j0 + w],
                                    vh_sb[:, kt, h, :],
                                    p_sb[:, j0:j0 + w],
                                    start=(kt == 0),
                                    stop=(kt == (qcp * QCP + QCP) // 128 - 1),
                                )
                    # normalization + output
                    for qcp in range(NQC):
                        stage = att.tile([VW, QCP], F32, tag="stage", bufs=2)
                        nc.vector.tensor_copy(
                            stage[DV:VW, :], pv_tiles[qcp][DV:VW, :]
                        )
                        s0 = att.tile([1, QCP], F32, tag="s0", bufs=2)
                        nc.sync.dma_start(s0[:], stage[DV:VW, :])
                        rec = att.tile([1, QCP], F32, tag="rec", bufs=2)
                        nc.vector.reciprocal_approx_fast(rec[:], s0[:])
                        rec2 = att.tile([1, QCP], F32, tag="rec2", bufs=2)
                        nc.vector.tensor_mul(
                            rec2[:],
                            rec[:],
                            qmask_sb[0:1, qcp * QCP:(qcp + 1) * QCP],
                        )
                        bcast = att.tile([DV, QCP], F32, tag="bcast", bufs=2)
                        nc.gpsimd.partition_broadcast(bcast[:], rec2[:])
                        o_sb = att.tile([DV, QCP], F32, tag="osb", bufs=2)
                        nc.vector.tensor_mul(
                            o_sb[:], pv_tiles[qcp][0:DV, :], bcast[:]
                        )
                        nc.sync.dma_start(
                            out[h * DV:(h + 1) * DV, qcp * QCP:(qcp + 1) * QCP],
                            o_sb[:, :],
                        )

    nc.compile()
    return nc


_NC = None


def _get_nc():
    global _NC
    if _NC is None:
        _NC = _build()
    return _NC


def _pack_kc(a):
    """[D, S]-like -> [128, KC*S] partition-major packing."""
    d, s = a.shape
    return np.ascontiguousarray(
        a.reshape(KC, 128, s).transpose(1, 0, 2).reshape(128, KC * s)
    )


def _make_in_maps(q, k, v, v_mask, q_mask, Wq, Wk, Wv):
    tri = np.zeros((128, 128), np.float32)
    kk, qq = np.meshgrid(np.arange(128), np.arange(128), indexing="ij")
    tri[qq >= kk] = 1.0

    in_maps = []
    for c in range(8):
        b, g = c // 4, c % 4
        cols = slice(g * GW, (g + 1) * GW)
        kb = np.where(v_mask[b] == 0, -np.float32(MAX), np.float32(0.0))
        kb = np.ascontiguousarray(kb.reshape(NKT, 128).T).astype(np.float32)
        in_maps.append({
            "qT": _pack_kc(np.ascontiguousarray(q[b].T)),
            "kT": _pack_kc(np.ascontiguousarray(k[b].T)),
            "vT": _pack_kc(np.ascontiguousarray(v[b].T)),
            "wq": _pack_kc(np.ascontiguousarray(Wq[:, cols])),
            "wk": _pack_kc(np.ascontiguousarray(Wk[:, cols])),
            "wv": _pack_kc(np.ascontiguousarray(Wv[:, cols])),
            "kbias": kb,
            "tri": tri,
            "qmask": q_mask[b].astype(np.float32).reshape(1, S),
        })
    return in_maps


def _ref_rows(q, k, v, v_mask, q_mask, Wq, Wk, Wv, b, r):
    """Reference (f32, numpy) for query rows [0, r) of batch b, all heads."""
    qh = (q[b, :r] @ Wq).reshape(r, H, DK).transpose(1, 0, 2)
    kh = (k[b] @ Wk).reshape(S, H, DK).transpose(1, 0, 2)
    vh = (v[b] @ Wv).reshape(S, H, DV).transpose(1, 0, 2)
    a = np.einsum("hqd,hkd->hqk", qh, kh) / np.float32(np.sqrt(DK))
    a = a - (1.0 - v_mask[b].astype(np.float32))[None, None, :] * np.float32(MAX)
    causal = np.tril(np.ones((r, S), np.float32), k=0)
    a = a - (1.0 - causal)[None, :, :] * np.float32(MAX)
    a = a - a.max(axis=-1, keepdims=True)
    e = np.exp(a)
    p = e / e.sum(axis=-1, keepdims=True)
    o = np.einsum("hqk,hkd->qhd", p, vh).reshape(r, H * DV)
    return o * q_mask[b, :r].astype(np.float32)[:, None]


def _run(q, k, v, v_mask, q_mask, Wq, Wk, Wv, trace=False):
    nc = _get_nc()
    in_maps = _make_in_maps(q, k, v, v_mask, q_mask, Wq, Wk, Wv)
    res = run_bass_kernel_spmd(nc, in_maps, core_ids=list(range(8)), trace=trace)

    out = np.zeros((B, S, H * DV), np.float32)
    for c in range(8):
        b, g = c // 4, c % 4
        out[b, :, g * GW:(g + 1) * GW] = res.results[c]["out"].T

    for b in range(B):
        nz = np.nonzero(v_mask[b])[0]
        r = int(nz[0]) if len(nz) else S
        if r > 0:
            out[b, :r, :] = _ref_rows(q, k, v, v_mask, q_mask, Wq, Wk, Wv, b, r)
    return out, res


def kernel(q, k, v, v_mask, q_mask, Wq, Wk, Wv):
    q = np.asarray(q, np.float32)
    k = np.asarray(k, np.float32)
    v = np.asarray(v, np.float32)
    v_mask = np.asarray(v_mask)
    q_mask = np.asarray(q_mask)
    Wq = np.asarray(Wq, np.float32)
    Wk = np.asarray(Wk, np.float32)
    Wv = np.asarray(Wv, np.float32)
    out, _ = _run(q, k, v, v_mask, q_mask, Wq, Wk, Wv, trace=False)
    return out


# revision 6
# speedup vs baseline: 1.8639x; 1.8639x over previous
"""Distributed multi-head causal attention for Trainium2 (8 NeuronCores).

Problem: nn_Attention (B=2, S=2048, D=1024, H=16, DK=DV=64), f32 inputs.

Sharding: batch x head-group. Core c handles batch b=c//4, heads 4*(c%4)..4*(c%4)+3.

Device algorithm (per core, bf16 matmuls with f32 PSUM accumulation):
  - project q/k/v against the core's weight-column slice: qhT/khT in
    [head-dim, seq] layout, vh in [seq, head-dim] layout with an appended
    ones-column (gives the softmax denominator for free during PV),
  - scoresT tiles [k-tile, q-chunk] = khT^T @ qhT (TensorE),
  - probs = Exp(scale*scores + pad_bias) on ScalarE (bias kills padded keys),
  - causal mask applied as a narrow per-key-tile "staircase" 0/1 multiply
    (DVE); tiles fully left of the staircase are skipped entirely,
  - PV accumulates vh_aug^T @ probsT into [65, q] PSUM (row 64 = denominator),
  - normalize: denominator row -> partition 0 (SBUF->SBUF DMA), fast
    reciprocal, partition-broadcast, elementwise multiply, DMA out.

Key optimization: the key-padding mask (v_mask) and query mask (q_mask) are
Bernoulli(1/2), and masked keys/queries contribute *exactly* zero in the
reference (exp(-1e10)=0 in f32; output rows are multiplied by q_mask). The
host therefore compacts both the key and query sequences to just the kept
positions (~halving each), which quarters the attention work. This is
numerically exact, not an approximation.

Host side: layout prep (transposes/slices/packing), compaction index maps,
staircase mask construction, output scatter, and patching of the
data-dependent degenerate rows (queries whose entire causal window is
key-masked; the reference's +/-1e10 additive-mask arithmetic makes those rows
attend uniformly to *future* unmasked keys, which the causal-skipping device
kernel intentionally does not compute).
"""

import numpy as np
import ml_dtypes

import concourse.bass as bass
import concourse.mybir as mybir
import concourse.tile as tile
from concourse import bacc
from concourse.bass_utils import run_bass_kernel_spmd

F32 = mybir.dt.float32
BF16 = mybir.dt.bfloat16

MAX = 1e10
B, S, D = 2, 2048, 1024
H, DK, DV = 16, 64, 64
HPC = 4            # heads per core
GW = HPC * DK      # 256: projected width per core
KC = D // 128      # 8 contraction chunks
VW = DV + 1        # 65: value dims + ones column


def _segs(off, end):
    """512-aligned segments of [off, end) — PSUM-bank-safe matmul pieces."""
    j = off
    while j < end:
        nxt = min(end, (j // 512 + 1) * 512)
        yield j, nxt - j
        j = nxt


def _build(cfg):
    nkt, nqp, wg, glo, qchunks, kt_last = (
        cfg["nkt"], cfg["nqp"], cfg["wg"], cfg["glo"], cfg["qchunks"],
        cfg["kt_last"])
    nkp = nkt * 128

    nc = bacc.Bacc("TRN2", target_bir_lowering=False, debug=False, num_devices=8)

    qT = nc.dram_tensor("qT", [128, KC * nqp], F32, kind="ExternalInput").ap()
    kT = nc.dram_tensor("kT", [128, KC * nkp], F32, kind="ExternalInput").ap()
    vT = nc.dram_tensor("vT", [128, KC * nkp], F32, kind="ExternalInput").ap()
    wq = nc.dram_tensor("wq", [128, KC * GW], F32, kind="ExternalInput").ap()
    wk = nc.dram_tensor("wk", [128, KC * GW], F32, kind="ExternalInput").ap()
    wv = nc.dram_tensor("wv", [128, KC * GW], F32, kind="ExternalInput").ap()
    kbias = nc.dram_tensor("kbias", [128, nkt], F32, kind="ExternalInput").ap()
    stair = nc.dram_tensor("stair", [128, nkt * wg], BF16, kind="ExternalInput").ap()
    out = nc.dram_tensor("out", [GW, nqp], F32, kind="ExternalOutput").ap()

    with tile.TileContext(nc) as tc:
        with tc.tile_pool(name="pers", bufs=1) as pers:
            qT_sb = pers.tile([128, KC, nqp], BF16)
            kT_sb = pers.tile([128, KC, nkp], BF16)
            vT_sb = pers.tile([128, KC, nkp], BF16)
            for kc in range(KC):
                nc.gpsimd.dma_start(qT_sb[:, kc, :], qT[:, kc * nqp:(kc + 1) * nqp])
                nc.gpsimd.dma_start(kT_sb[:, kc, :], kT[:, kc * nkp:(kc + 1) * nkp])
                nc.gpsimd.dma_start(vT_sb[:, kc, :], vT[:, kc * nkp:(kc + 1) * nkp])
            wq_sb = pers.tile([128, KC, GW], BF16)
            nc.gpsimd.dma_start(wq_sb[:], wq[:, :].rearrange("p (kc n) -> p kc n", kc=KC))
            wk_sb = pers.tile([128, KC, GW], BF16)
            nc.gpsimd.dma_start(wk_sb[:], wk[:, :].rearrange("p (kc n) -> p kc n", kc=KC))
            wv_sb = pers.tile([128, KC, GW], BF16)
            nc.gpsimd.dma_start(wv_sb[:], wv[:, :].rearrange("p (kc n) -> p kc n", kc=KC))
            kbias_sb = pers.tile([128, nkt], F32)
            nc.sync.dma_start(kbias_sb[:], kbias[:, :])
            stair_sb = pers.tile([128, nkt, wg], BF16)
            nc.sync.dma_start(
                stair_sb[:], stair[:, :].rearrange("p (kt w) -> p kt w", kt=nkt))

            qhT_sb = pers.tile([128, 2, nqp], BF16)   # [p, m, s]: qh[s, m*128+p]
            khT_sb = pers.tile([128, 2, nkp], BF16)
            vh_sb = pers.tile([128, nkt, HPC, VW], BF16)  # col DV = ones
            nc.gpsimd.memset(vh_sb[:, :, :, DV:VW], 1.0)

            # ---- q/k projections ----
            with tc.tile_pool(name="ps_qk", bufs=8, space="PSUM") as ps_qk:
                for nm, w_sb, src_sb, dst_sb, nn in (
                    ("q", wq_sb, qT_sb, qhT_sb, nqp),
                    ("k", wk_sb, kT_sb, khT_sb, nkp),
                ):
                    chunks = list(_segs(0, nn))
                    tiles = {}
                    for m in range(2):
                        for ci in range(len(chunks)):
                            tiles[(m, ci)] = ps_qk.tile(
                                [128, 512], F32, tag="qk", name=f"qkps_{nm}_{m}_{ci}")
                    for kc in range(KC):
                        for m in range(2):
                            for ci, (c0, cw) in enumerate(chunks):
                                nc.tensor.matmul(
                                    tiles[(m, ci)][:, 0:cw],
                                    w_sb[:, kc, m * 128:(m + 1) * 128],
                                    src_sb[:, kc, c0:c0 + cw],
                                    start=(kc == 0),
                                    stop=(kc == KC - 1),
                                )
                    for m in range(2):
                        for ci, (c0, cw) in enumerate(chunks):
                            nc.vector.tensor_copy(
                                dst_sb[:, m, c0:c0 + cw], tiles[(m, ci)][:, 0:cw])

            # ---- v projection ----
            with tc.tile_pool(name="ps_v", bufs=4, space="PSUM") as ps_v:
                for st in range(nkt):
                    pvp = ps_v.tile([128, GW], F32, tag="v", name=f"vps_{st}")
                    for kc in range(KC):
                        nc.tensor.matmul(
                            pvp[:],
                            vT_sb[:, kc, st * 128:(st + 1) * 128],
                            wv_sb[:, kc, :],
                            start=(kc == 0),
                            stop=(kc == KC - 1),
                        )
                    nc.vector.tensor_copy(
                        vh_sb[:, st, :, 0:DV],
                        pvp[:].rearrange("p (h d) -> p h d", d=DV),
                    )

            # ---- attention ----
            with (
                tc.tile_pool(name="ps_s", bufs=2, space="PSUM") as ps_s,
                tc.tile_pool(name="ps_pv", bufs=1, space="PSUM") as ps_pv,
                tc.tile_pool(name="att", bufs=1) as att,
            ):
                for h in range(HPC):
                    p0 = (h % 2) * 64
                    m = h // 2
                    pv_tiles = [
                        ps_pv.tile([VW, cw], F32, tag=f"pv{ci}", name=f"pvps_{h}_{ci}")
                        for ci, (c0, cw) in enumerate(qchunks)
                    ]
                    for kt in range(nkt):
                        for ci, (c0, cw) in enumerate(qchunks):
                            if glo[kt] >= c0 + cw:
                                continue
                            off = max(0, glo[kt] - c0)
                            s_ps = ps_s.tile([128, 1024], F32, tag="s", name="s_ps")
                            for j0, w in _segs(off, cw):
                                nc.tensor.matmul(
                                    s_ps[:, j0:j0 + w],
                                    khT_sb[p0:p0 + 64, m, kt * 128:(kt + 1) * 128],
                                    qhT_sb[p0:p0 + 64, m, c0 + j0:c0 + j0 + w],
                                    start=True,
                                    stop=True,
                                )
                            p_sb = att.tile([128, 1024], BF16, tag="probs",
                                            bufs=4, name="p_sb")
                            nc.scalar.activation(
                                p_sb[:, off:cw],
                                s_ps[:, off:cw],
                                mybir.ActivationFunctionType.Exp,
                                bias=kbias_sb[:, kt:kt + 1],
                                scale=float(1.0 / np.sqrt(DK)),
                            )
                            # staircase causal mask on [glo, glo+wg) overlap
                            a = max(glo[kt], c0)
                            bb = min(glo[kt] + wg, c0 + cw)
                            if a < bb:
                                nc.vector.tensor_mul(
                                    p_sb[:, a - c0:bb - c0],
                                    p_sb[:, a - c0:bb - c0],
                                    stair_sb[:, kt, a - glo[kt]:bb - glo[kt]],
                                )
                            for j0, w in _segs(off, cw):
                                nc.tensor.matmul(
                                    pv_tiles[ci][:, j0:j0 + w],
                                    vh_sb[:, kt, h, :],
                                    p_sb[:, j0:j0 + w],
                                    start=(kt == 0),
                                    stop=(kt == kt_last[ci]),
                                )
                    # normalization + output
                    for ci, (c0, cw) in enumerate(qchunks):
                        stg = att.tile([VW, cw], F32, tag=f"stage{ci}",
                                       bufs=2, name="stg")
                        nc.vector.tensor_copy(stg[DV:VW, :], pv_tiles[ci][DV:VW, :])
                        s0 = att.tile([1, cw], F32, tag=f"s0{ci}", bufs=2, name="s0")
                        nc.sync.dma_start(s0[:], stg[DV:VW, :])
                        rec = att.tile([1, cw], F32, tag=f"rec{ci}", bufs=2, name="rec")
                        nc.vector.reciprocal_approx_fast(rec[:], s0[:])
                        bcast = att.tile([DV, cw], F32, tag=f"bcast{ci}",
                                         bufs=2, name="bcast")
                        nc.gpsimd.partition_broadcast(bcast[:], rec[:])
                        o_sb = att.tile([DV, cw], F32, tag=f"osb{ci}",
                                        bufs=2, name="o_sb")
                        nc.vector.tensor_mul(o_sb[:], pv_tiles[ci][0:DV, :], bcast[:])
                        nc.sync.dma_start(
                            out[h * DV:(h + 1) * DV, c0:c0 + cw], o_sb[:, :])

    nc.compile()
    return nc


_NC_CACHE = {}


def _get_nc(cfg):
    key = (cfg["nkt"], cfg["nqp"], cfg["wg"], cfg["glo"],
           tuple(cfg["qchunks"]), tuple(cfg["kt_last"]))
    if key not in _NC_CACHE:
        _NC_CACHE[key] = _build(cfg)
    return _NC_CACHE[key]


def _pack_kc(a):
    """[D, N]-like -> [128, KC*N] partition-major packing."""
    d, n = a.shape
    return np.ascontiguousarray(
        a.reshape(KC, 128, n).transpose(1, 0, 2).reshape(128, KC * n)
    )


def _plan(v_mask, q_mask):
    """Compaction plan shared by all cores (shapes must be SPMD-uniform)."""
    keep_k = [np.nonzero(v_mask[b])[0] for b in range(B)]
    keep_q = [np.nonzero(q_mask[b])[0] for b in range(B)]
    nkp = ((max(len(x) for x in keep_k) + 127) // 128) * 128
    nqp = ((max(len(x) for x in keep_q) + 63) // 64) * 64
    nkt = nkp // 128

    # per-batch causal boundaries c_j: first compact-q column with Q >= K_j
    cbs = []
    for b in range(B):
        K = np.full(nkp, -1, np.int64)          # pads: allowed everywhere
        K[:len(keep_k[b])] = keep_k[b]
        Q = np.full(nqp, S + nqp, np.int64)     # pads: later than everything
        Q[:len(keep_q[b])] = keep_q[b]
        cbs.append(np.searchsorted(Q, K))       # [nkp]
    cbs = np.stack(cbs)                          # [B, nkp]

    cb_t = cbs.reshape(B, nkt, 128)
    glo = tuple(int(x) & ~7 for x in cb_t.min(axis=(0, 2)))
    hi = cb_t.max(axis=(0, 2))
    wg = int((int((hi - np.array(glo)).max()) + 63) // 64) * 64
    wg = max(wg, 64)

    qchunks = []
    c0 = 0
    while c0 < nqp:
        cw = min(1024, nqp - c0)
        qchunks.append((c0, cw))
        c0 += cw
    kt_last = [max(kt for kt in range(nkt) if glo[kt] < c0 + cw)
               for (c0, cw) in qchunks]

    cfg = dict(nkt=nkt, nqp=nqp, wg=wg, glo=glo, qchunks=qchunks,
               kt_last=tuple(kt_last))
    return cfg, keep_k, keep_q, cbs


def _make_in_maps(q, k, v, v_mask, q_mask, Wq, Wk, Wv, cfg, keep_k, keep_q, cbs):
    nkt, nqp, wg, glo = cfg["nkt"], cfg["nqp"], cfg["wg"], cfg["glo"]
    nkp = nkt * 128

    per_batch = []
    for b in range(B):
        kk, kq = keep_k[b], keep_q[b]

        def compact(x, keep, n):
            xt = x[b].T  # [D, S]
            outa = np.zeros((D, n), np.float32)
            outa[:, :len(keep)] = xt[:, keep]
            return _pack_kc(outa)

        kb = np.zeros((128, nkt), np.float32)
        kb_flat = np.zeros(nkp, np.float32)
        kb_flat[len(kk):] = -np.float32(MAX)
        kb[:] = kb_flat.reshape(nkt, 128).T

        # staircase masks [128, nkt, wg]: 1 iff column (glo[kt]+w) >= c_j
        st = np.zeros((128, nkt, wg), ml_dtypes.bfloat16)
        for kt in range(nkt):
            c = cbs[b, kt * 128:(kt + 1) * 128]          # [128]
            w = glo[kt] + np.arange(wg)                   # [wg]
            st[:, kt, :] = (w[None, :] >= c[:, None]).astype(ml_dtypes.bfloat16)

        per_batch.append(dict(
            qT=compact(q, kq, nqp), kT=compact(k, kk, nkp), vT=compact(v, kk, nkp),
            kbias=np.ascontiguousarray(kb),
            stair=np.ascontiguousarray(st.reshape(128, nkt * wg)),
        ))

    in_maps = []
    for c in range(8):
        b, g = c // 4, c % 4
        cols = slice(g * GW, (g + 1) * GW)
        m = dict(per_batch[b])
        m["wq"] = _pack_kc(np.ascontiguousarray(Wq[:, cols]))
        m["wk"] = _pack_kc(np.ascontiguousarray(Wk[:, cols]))
        m["wv"] = _pack_kc(np.ascontiguousarray(Wv[:, cols]))
        in_maps.append(m)
    return in_maps


def _ref_rows(q, k, v, v_mask, q_mask, Wq, Wk, Wv, b, r):
    """Reference (f32, numpy) for query rows [0, r) of batch b, all heads."""
    qh = (q[b, :r] @ Wq).reshape(r, H, DK).transpose(1, 0, 2)
    kh = (k[b] @ Wk).reshape(S, H, DK).transpose(1, 0, 2)
    vh = (v[b] @ Wv).reshape(S, H, DV).transpose(1, 0, 2)
    a = np.einsum("hqd,hkd->hqk", qh, kh) / np.float32(np.sqrt(DK))
    a = a - (1.0 - v_mask[b].astype(np.float32))[None, None, :] * np.float32(MAX)
    causal = np.tril(np.ones((r, S), np.float32), k=0)
    a = a - (1.0 - causal)[None, :, :] * np.float32(MAX)
    a = a - a.max(axis=-1, keepdims=True)
    e = np.exp(a)
    p = e / e.sum(axis=-1, keepdims=True)
    o = np.einsum("hqk,hkd->qhd", p, vh).reshape(r, H * DV)
    return o * q_mask[b, :r].astype(np.float32)[:, None]


def _run(q, k, v, v_mask, q_mask, Wq, Wk, Wv, trace=False):
    cfg, keep_k, keep_q, cbs = _plan(v_mask, q_mask)
    nc = _get_nc(cfg)
    in_maps = _make_in_maps(q, k, v, v_mask, q_mask, Wq, Wk, Wv,
                            cfg, keep_k, keep_q, cbs)
    res = run_bass_kernel_spmd(nc, in_maps, core_ids=list(range(8)), trace=trace)

    out = np.zeros((B, S, H * DV), np.float32)
    for c in range(8):
        b, g = c // 4, c % 4
        kq = keep_q[b]
        out[b, kq, g * GW:(g + 1) * GW] = res.results[c]["out"][:, :len(kq)].T

    for b in range(B):
        nz = np.nonzero(v_mask[b])[0]
        r = int(nz[0]) if len(nz) else S
        if r > 0:
            out[b, :r, :] = _ref_rows(q, k, v, v_mask, q_mask, Wq, Wk, Wv, b, r)
    return out, res


def kernel(q, k, v, v_mask, q_mask, Wq, Wk, Wv):
    q = np.asarray(q, np.float32)
    k = np.asarray(k, np.float32)
    v = np.asarray(v, np.float32)
    v_mask = np.asarray(v_mask)
    q_mask = np.asarray(q_mask)
    Wq = np.asarray(Wq, np.float32)
    Wk = np.asarray(Wk, np.float32)
    Wv = np.asarray(Wv, np.float32)
    out, _ = _run(q, k, v, v_mask, q_mask, Wq, Wk, Wv, trace=False)
    return out


# revision 7
# speedup vs baseline: 2.0716x; 1.1114x over previous
"""Distributed multi-head causal attention for Trainium2 (8 NeuronCores).

Problem: nn_Attention (B=2, S=2048, D=1024, H=16, DK=DV=64), f32 inputs.

Sharding: batch x head-group. Core c handles batch b=c//4, heads 4*(c%4)..4*(c%4)+3.

Device algorithm (per core, bf16 matmuls with f32 PSUM accumulation):
  - project q/k/v against the core's weight-column slice: qhT/khT in
    [head-dim, seq] layout, vh in [seq, head-dim] layout with an appended
    ones-column (gives the softmax denominator for free during PV),
  - scoresT tiles [k-tile, q-chunk] = khT^T @ qhT (TensorE),
  - probs = Exp(scale*scores + pad_bias) on ScalarE (bias kills padded keys),
  - causal mask applied as a narrow per-key-tile "staircase" 0/1 multiply
    (DVE); tiles fully left of the staircase are skipped entirely,
  - PV accumulates vh_aug^T @ probsT into [65, q] PSUM (row 64 = denominator),
  - normalize: denominator row -> partition 0 (SBUF->SBUF DMA), fast
    reciprocal, partition-broadcast, elementwise multiply, DMA out.

Key optimization: the key-padding mask (v_mask) and query mask (q_mask) are
Bernoulli(1/2), and masked keys/queries contribute *exactly* zero in the
reference (exp(-1e10)=0 in f32; output rows are multiplied by q_mask). The
host therefore compacts both the key and query sequences to just the kept
positions (~halving each), which quarters the attention work. This is
numerically exact, not an approximation.

Host side: layout prep (transposes/slices/packing), compaction index maps,
staircase mask construction, output scatter, and patching of the
data-dependent degenerate rows (queries whose entire causal window is
key-masked; the reference's +/-1e10 additive-mask arithmetic makes those rows
attend uniformly to *future* unmasked keys, which the causal-skipping device
kernel intentionally does not compute).
"""

import numpy as np
import ml_dtypes

import concourse.bass as bass
import concourse.mybir as mybir
import concourse.tile as tile
from concourse import bacc
from concourse.bass_utils import run_bass_kernel_spmd

F32 = mybir.dt.float32
BF16 = mybir.dt.bfloat16

MAX = 1e10
B, S, D = 2, 2048, 1024
H, DK, DV = 16, 64, 64
HPC = 4            # heads per core
GW = HPC * DK      # 256: projected width per core
KC = D // 128      # 8 contraction chunks
VW = DV + 1        # 65: value dims + ones column


def _segs(off, end):
    """512-aligned segments of [off, end) — PSUM-bank-safe matmul pieces."""
    j = off
    while j < end:
        nxt = min(end, (j // 512 + 1) * 512)
        yield j, nxt - j
        j = nxt


def _build(cfg):
    nkt, nqp, wg, glo = cfg["nkt"], cfg["nqp"], cfg["wg"], cfg["glo"]
    nkp = nkt * 128
    scale = float(1.0 / np.sqrt(DK))

    # PV/probs/norm chunking: one main chunk (<=1024) + small tail
    cw_main = min(1024, nqp)
    chunks = [(0, cw_main)]
    if nqp > 1024:
        chunks.append((1024, nqp - 1024))
    # scores sub-chunks (one PSUM bank each)
    subs = {}
    for (c0, cw) in chunks:
        subs[(c0, cw)] = list((c0 + j, w) for j, w in _segs(0, cw))
    kt_last = {c: max(kt for kt in range(nkt) if glo[kt] < c[0] + c[1])
               for c in chunks}
    # k-column blocks for the DMA/proj/attention pipeline
    kblocks = list(_segs(0, nkp))

    nc = bacc.Bacc("TRN2", target_bir_lowering=False, debug=False, num_devices=8)

    qT = nc.dram_tensor("qT", [128, KC * nqp], F32, kind="ExternalInput").ap()
    kT = nc.dram_tensor("kT", [128, KC * nkp], F32, kind="ExternalInput").ap()
    vT = nc.dram_tensor("vT", [128, KC * nkp], F32, kind="ExternalInput").ap()
    wq = nc.dram_tensor("wq", [128, KC * GW], F32, kind="ExternalInput").ap()
    wk = nc.dram_tensor("wk", [128, KC * GW], F32, kind="ExternalInput").ap()
    wv = nc.dram_tensor("wv", [128, KC * GW], F32, kind="ExternalInput").ap()
    kbias = nc.dram_tensor("kbias", [128, nkt], F32, kind="ExternalInput").ap()
    stair = nc.dram_tensor("stair", [128, nkt * wg], BF16, kind="ExternalInput").ap()
    out = nc.dram_tensor("out", [GW, nqp], F32, kind="ExternalOutput").ap()

    with tile.TileContext(nc) as tc:
        with tc.tile_pool(name="pers", bufs=1) as pers:
            # --- DMA issue order matters: weights first, then qT (q-proj
            # pipelines under it), then per-k-block vT/kT (v/k-proj + pass-A
            # attention pipeline under those). All casting loads share the
            # single SWDGE queue and complete in this order.
            wq_sb = pers.tile([128, KC, GW], BF16)
            nc.gpsimd.dma_start(wq_sb[:], wq[:, :].rearrange("p (kc n) -> p kc n", kc=KC))
            wk_sb = pers.tile([128, KC, GW], BF16)
            nc.gpsimd.dma_start(wk_sb[:], wk[:, :].rearrange("p (kc n) -> p kc n", kc=KC))
            wv_sb = pers.tile([128, KC, GW], BF16)
            nc.gpsimd.dma_start(wv_sb[:], wv[:, :].rearrange("p (kc n) -> p kc n", kc=KC))

            qT_sb = pers.tile([128, KC, nqp], BF16)
            for kc in range(KC):
                nc.gpsimd.dma_start(qT_sb[:, kc, :], qT[:, kc * nqp:(kc + 1) * nqp])
            kT_sb = pers.tile([128, KC, nkp], BF16)
            vT_sb = pers.tile([128, KC, nkp], BF16)
            kT_r = kT[:, :].rearrange("p (kc n) -> p kc n", kc=KC)
            vT_r = vT[:, :].rearrange("p (kc n) -> p kc n", kc=KC)
            for (b0, bw) in kblocks:
                nc.gpsimd.dma_start(vT_sb[:, :, b0:b0 + bw], vT_r[:, :, b0:b0 + bw])
                nc.gpsimd.dma_start(kT_sb[:, :, b0:b0 + bw], kT_r[:, :, b0:b0 + bw])

            kbias_sb = pers.tile([128, nkt], F32)
            nc.sync.dma_start(kbias_sb[:], kbias[:, :])
            stair_sb = pers.tile([128, nkt, wg], BF16)
            nc.sync.dma_start(
                stair_sb[:], stair[:, :].rearrange("p (kt w) -> p kt w", kt=nkt))

            qhT_sb = pers.tile([128, 2, nqp], BF16)   # [p, m, s]: qh[s, m*128+p]
            khT_sb = pers.tile([128, 2, nkp], BF16)
            vh_sb = pers.tile([128, nkt, HPC, VW], BF16)  # col DV = ones
            nc.gpsimd.memset(vh_sb[:, :, :, DV:VW], 1.0)

            with (
                tc.tile_pool(name="ps_pj", bufs=2, space="PSUM") as ps_pj,
                tc.tile_pool(name="ps_s", bufs=2, space="PSUM") as ps_s,
                tc.tile_pool(name="ps_pv", bufs=2, space="PSUM") as ps_pv,
                tc.tile_pool(name="att", bufs=1) as att,
            ):
                # ---- q projection (pipelines under the qT load) ----
                for m in range(2):
                    for (c0, cw) in _segs(0, nqp):
                        pj = ps_pj.tile([128, 512], F32, tag="pj", name="pj_q")
                        for kc in range(KC):
                            nc.tensor.matmul(
                                pj[:, 0:cw],
                                wq_sb[:, kc, m * 128:(m + 1) * 128],
                                qT_sb[:, kc, c0:c0 + cw],
                                start=(kc == 0), stop=(kc == KC - 1))
                        nc.vector.tensor_copy(qhT_sb[:, m, c0:c0 + cw], pj[:, 0:cw])

                def kv_proj(b0, bw):
                    # v-proj for seq tiles in the block
                    for st in range(b0 // 128, (b0 + bw) // 128):
                        pj = ps_pj.tile([128, GW], F32, tag="pj", name="pj_v")
                        for kc in range(KC):
                            nc.tensor.matmul(
                                pj[:],
                                vT_sb[:, kc, st * 128:(st + 1) * 128],
                                wv_sb[:, kc, :],
                                start=(kc == 0), stop=(kc == KC - 1))
                        nc.vector.tensor_copy(
                            vh_sb[:, st, :, 0:DV],
                            pj[:].rearrange("p (h d) -> p h d", d=DV))
                    # k-proj m0/m1 for the block
                    for m in range(2):
                        pj = ps_pj.tile([128, 512], F32, tag="pj", name="pj_k")
                        for kc in range(KC):
                            nc.tensor.matmul(
                                pj[:, 0:bw],
                                wk_sb[:, kc, m * 128:(m + 1) * 128],
                                kT_sb[:, kc, b0:b0 + bw],
                                start=(kc == 0), stop=(kc == KC - 1))
                        nc.vector.tensor_copy(khT_sb[:, m, b0:b0 + bw], pj[:, 0:bw])

                def attention(kt, h, pv_tiles):
                    p0 = (h % 2) * 64
                    m = h // 2
                    for ci, (c0, cw) in enumerate(chunks):
                        if glo[kt] >= c0 + cw:
                            continue
                        off = max(0, glo[kt] - c0)
                        p_sb = att.tile([128, 1024], BF16, tag="probs",
                                        bufs=6, name="p_sb")
                        for (s0_, sw) in subs[(c0, cw)]:
                            if glo[kt] >= s0_ + sw:
                                continue
                            soff = max(0, glo[kt] - s0_)
                            s_ps = ps_s.tile([128, 512], F32, tag="s", name="s_ps")
                            nc.tensor.matmul(
                                s_ps[:, soff:sw],
                                khT_sb[p0:p0 + 64, m, kt * 128:(kt + 1) * 128],
                                qhT_sb[p0:p0 + 64, m, s0_ + soff:s0_ + sw],
                                start=True, stop=True)
                            nc.scalar.activation(
                                p_sb[:, s0_ - c0 + soff:s0_ - c0 + sw],
                                s_ps[:, soff:sw],
                                mybir.ActivationFunctionType.Exp,
                                bias=kbias_sb[:, kt:kt + 1],
                                scale=scale)
                        a = max(glo[kt], c0)
                        bb = min(glo[kt] + wg, c0 + cw)
                        if a < bb:
                            nc.vector.tensor_mul(
                                p_sb[:, a - c0:bb - c0],
                                p_sb[:, a - c0:bb - c0],
                                stair_sb[:, kt, a - glo[kt]:bb - glo[kt]])
                        for j0, w in _segs(off, cw):
                            nc.tensor.matmul(
                                pv_tiles[ci][:, j0:j0 + w],
                                vh_sb[:, kt, h, :],
                                p_sb[:, j0:j0 + w],
                                start=(kt == 0),
                                stop=(kt == kt_last[(c0, cw)]))

                def norm_pass(heads, pv_all):
                    stg = att.tile([VW, 2 * nqp], F32, tag="stg", bufs=2, name="stg")
                    for i, h in enumerate(heads):
                        for ci, (c0, cw) in enumerate(chunks):
                            nc.vector.tensor_copy(
                                stg[DV:VW, i * nqp + c0:i * nqp + c0 + cw],
                                pv_all[h][ci][DV:VW, :])
                    sden = att.tile([1, 2 * nqp], F32, tag="sden", bufs=2, name="sden")
                    nc.sync.dma_start(sden[:], stg[DV:VW, :])
                    rec = att.tile([1, 2 * nqp], F32, tag="rec", bufs=2, name="rec")
                    nc.vector.reciprocal_approx_fast(rec[:], sden[:])
                    bcast = att.tile([DV, 2 * nqp], F32, tag="bcast", bufs=2,
                                     name="bcast")
                    nc.gpsimd.partition_broadcast(bcast[:], rec[:])
                    for i, h in enumerate(heads):
                        for ci, (c0, cw) in enumerate(chunks):
                            o_sb = att.tile([DV, cw], F32, tag=f"osb{ci}",
                                            bufs=2, name="o_sb")
                            nc.vector.tensor_mul(
                                o_sb[:], pv_all[h][ci][0:DV, :],
                                bcast[:, i * nqp + c0:i * nqp + c0 + cw])
                            nc.sync.dma_start(
                                out[h * DV:(h + 1) * DV, c0:c0 + cw], o_sb[:, :])

                # ---- pass A: heads 0,1 pipelined with k/v loads+projections ----
                pv_all = {}
                for h in (0, 1):
                    pv_all[h] = [
                        ps_pv.tile([VW, cw], F32, tag=f"pv{ci}", name=f"pv_{h}_{ci}")
                        for ci, (c0, cw) in enumerate(chunks)]
                for (b0, bw) in kblocks:
                    kv_proj(b0, bw)
                    for kt in range(b0 // 128, (b0 + bw) // 128):
                        for h in (0, 1):
                            attention(kt, h, pv_all[h])
                norm_pass((0, 1), pv_all)

                # ---- pass B: heads 2,3 (everything resident) ----
                pv_all = {}
                for h in (2, 3):
                    pv_all[h] = [
                        ps_pv.tile([VW, cw], F32, tag=f"pv{ci}", name=f"pv_{h}_{ci}")
                        for ci, (c0, cw) in enumerate(chunks)]
                for kt in range(nkt):
                    for h in (2, 3):
                        attention(kt, h, pv_all[h])
                norm_pass((2, 3), pv_all)

    nc.compile()
    return nc


_NC_CACHE = {}


def _get_nc(cfg):
    key = (cfg["nkt"], cfg["nqp"], cfg["wg"], cfg["glo"])
    if key not in _NC_CACHE:
        _NC_CACHE[key] = _build(cfg)
    return _NC_CACHE[key]


def _pack_kc(a):
    """[D, N]-like -> [128, KC*N] partition-major packing."""
    d, n = a.shape
    return np.ascontiguousarray(
        a.reshape(KC, 128, n).transpose(1, 0, 2).reshape(128, KC * n)
    )


def _plan(v_mask, q_mask):
    """Compaction plan shared by all cores (shapes must be SPMD-uniform)."""
    keep_k = [np.nonzero(v_mask[b])[0] for b in range(B)]
    keep_q = [np.nonzero(q_mask[b])[0] for b in range(B)]
    nkp = ((max(len(x) for x in keep_k) + 127) // 128) * 128
    nqp = ((max(len(x) for x in keep_q) + 63) // 64) * 64
    nkt = nkp // 128

    # per-batch causal boundaries c_j: first compact-q column with Q >= K_j
    cbs = []
    for b in range(B):
        K = np.full(nkp, -1, np.int64)          # pads: allowed everywhere
        K[:len(keep_k[b])] = keep_k[b]
        Q = np.full(nqp, S + nqp, np.int64)     # pads: later than everything
        Q[:len(keep_q[b])] = keep_q[b]
        cbs.append(np.searchsorted(Q, K))       # [nkp]
    cbs = np.stack(cbs)                          # [B, nkp]

    cb_t = cbs.reshape(B, nkt, 128)
    glo = tuple(int(x) & ~7 for x in cb_t.min(axis=(0, 2)))
    hi = cb_t.max(axis=(0, 2))
    wg = int((int((hi - np.array(glo)).max()) + 63) // 64) * 64
    wg = max(wg, 64)

    cfg = dict(nkt=nkt, nqp=nqp, wg=wg, glo=glo)
    return cfg, keep_k, keep_q, cbs


def _make_in_maps(q, k, v, v_mask, q_mask, Wq, Wk, Wv, cfg, keep_k, keep_q, cbs):
    nkt, nqp, wg, glo = cfg["nkt"], cfg["nqp"], cfg["wg"], cfg["glo"]
    nkp = nkt * 128

    per_batch = []
    for b in range(B):
        kk, kq = keep_k[b], keep_q[b]

        def compact(x, keep, n):
            xt = x[b].T  # [D, S]
            outa = np.zeros((D, n), np.float32)
            outa[:, :len(keep)] = xt[:, keep]
            return _pack_kc(outa)

        kb = np.zeros((128, nkt), np.float32)
        kb_flat = np.zeros(nkp, np.float32)
        kb_flat[len(kk):] = -np.float32(MAX)
        kb[:] = kb_flat.reshape(nkt, 128).T

        # staircase masks [128, nkt, wg]: 1 iff column (glo[kt]+w) >= c_j
        st = np.zeros((128, nkt, wg), ml_dtypes.bfloat16)
        for kt in range(nkt):
            c = cbs[b, kt * 128:(kt + 1) * 128]          # [128]
            w = glo[kt] + np.arange(wg)                   # [wg]
            st[:, kt, :] = (w[None, :] >= c[:, None]).astype(ml_dtypes.bfloat16)

        per_batch.append(dict(
            qT=compact(q, kq, nqp), kT=compact(k, kk, nkp), vT=compact(v, kk, nkp),
            kbias=np.ascontiguousarray(kb),
            stair=np.ascontiguousarray(st.reshape(128, nkt * wg)),
        ))

    in_maps = []
    for c in range(8):
        b, g = c // 4, c % 4
        cols = slice(g * GW, (g + 1) * GW)
        m = dict(per_batch[b])
        m["wq"] = _pack_kc(np.ascontiguousarray(Wq[:, cols]))
        m["wk"] = _pack_kc(np.ascontiguousarray(Wk[:, cols]))
        m["wv"] = _pack_kc(np.ascontiguousarray(Wv[:, cols]))
        in_maps.append(m)
    return in_maps


def _ref_rows(q, k, v, v_mask, q_mask, Wq, Wk, Wv, b, r):
    """Reference (f32, numpy) for query rows [0, r) of batch b, all heads."""
    qh = (q[b, :r] @ Wq).reshape(r, H, DK).transpose(1, 0, 2)
    kh = (k[b] @ Wk).reshape(S, H, DK).transpose(1, 0, 2)
    vh = (v[b] @ Wv).reshape(S, H, DV).transpose(1, 0, 2)
    a = np.einsum("hqd,hkd->hqk", qh, kh) / np.float32(np.sqrt(DK))
    a = a - (1.0 - v_mask[b].astype(np.float32))[None, None, :] * np.float32(MAX)
    causal = np.tril(np.ones((r, S), np.float32), k=0)
    a = a - (1.0 - causal)[None, :, :] * np.float32(MAX)
    a = a - a.max(axis=-1, keepdims=True)
    e = np.exp(a)
    p = e / e.sum(axis=-1, keepdims=True)
    o = np.einsum("hqk,hkd->qhd", p, vh).reshape(r, H * DV)
    return o * q_mask[b, :r].astype(np.float32)[:, None]


def _run(q, k, v, v_mask, q_mask, Wq, Wk, Wv, trace=False):
    cfg, keep_k, keep_q, cbs = _plan(v_mask, q_mask)
    nc = _get_nc(cfg)
    in_maps = _make_in_maps(q, k, v, v_mask, q_mask, Wq, Wk, Wv,
                            cfg, keep_k, keep_q, cbs)
    res = run_bass_kernel_spmd(nc, in_maps, core_ids=list(range(8)), trace=trace)

    out = np.zeros((B, S, H * DV), np.float32)
    for c in range(8):
        b, g = c // 4, c % 4
        kq = keep_q[b]
        out[b, kq, g * GW:(g + 1) * GW] = res.results[c]["out"][:, :len(kq)].T

    for b in range(B):
        nz = np.nonzero(v_mask[b])[0]
        r = int(nz[0]) if len(nz) else S
        if r > 0:
            out[b, :r, :] = _ref_rows(q, k, v, v_mask, q_mask, Wq, Wk, Wv, b, r)
    return out, res


def kernel(q, k, v, v_mask, q_mask, Wq, Wk, Wv):
    q = np.asarray(q, np.float32)
    k = np.asarray(k, np.float32)
    v = np.asarray(v, np.float32)
    v_mask = np.asarray(v_mask)
    q_mask = np.asarray(q_mask)
    Wq = np.asarray(Wq, np.float32)
    Wk = np.asarray(Wk, np.float32)
    Wv = np.asarray(Wv, np.float32)
    out, _ = _run(q, k, v, v_mask, q_mask, Wq, Wk, Wv, trace=False)
    return out
